# revision 1
# baseline (speedup 1.0000x reference)
"""Trainium2 Bass kernel for nn_ConditionalSpline1DFlow (K=16 RQS flow).

Data-parallel over 8 cores (B=4096 -> 512 rows/core). Per core:
  1. Conditioner MLP on TensorE (feature-major).
  2. Spline params per row; rescale bin k's rational-quadratic by
     s_k = delta_0/delta_k so numerator N, denominator D and
     derivative-numerator C become globally CONTINUOUS piecewise
     quadratics in x.
  3. Evaluate N, D, C gather-free in the clipped-ramp basis
        P(x) = const + sum_k a_k*(t_k - x_k)^2 + b_k*(t_k - x_k),
        t_k = clip(x, x_k, x_{k+1})
     on TensorE: rows packed (b*16+k) so one [128, 24] matmul contracts
     all 16 bins x 3 polys for 8 batch rows at once; PSUM accumulates the
     (linear, square) stream pair.
  4. out = N/D + (y - clip(y)); logdet = (ln C - 2 ln D) * (y == clip(y)).
"""
import sys
import numpy as np

K = 16
BOUND = 5.0
MBW = 1e-3
MBH = 1e-3
MD = 1e-3
B_FULL, N = 4096, 1024
CD, H = 64, 256
OUT3 = 3 * K + 1
NCORES = 8
BL = B_FULL // NCORES   # 512 rows per core
T = BL // 128           # 4 partition tiles
G = 128 // 8            # (unused) 8-row groups
GG = 128 // 16          # 8 groups of 16 rows per tile
CH = N // 512           # 2 free-dim chunks

MODE = "t"  # "t": stream clipped-t w/ folded consts; "u": stream t - x_k

_CACHE = {}


def _ensure_path():
    for p in ("/opt/trn_rl_repo",):
        if p not in sys.path:
            sys.path.insert(0, p)


def _build_nc():
    _ensure_path()
    import concourse.bacc as bacc
    import concourse.tile as tile
    from concourse import mybir

    fp32 = mybir.dt.float32
    nc = bacc.Bacc("TRN2", target_bir_lowering=False, debug=False)

    io = dict(
        cond=nc.dram_tensor("cond", [BL, CD], fp32, kind="ExternalInput"),
        y=nc.dram_tensor("y", [BL, N], fp32, kind="ExternalInput"),
        W1=nc.dram_tensor("W1", [CD, H], fp32, kind="ExternalInput"),
        b1=nc.dram_tensor("b1", [H], fp32, kind="ExternalInput"),
        W2=nc.dram_tensor("W2", [H, H], fp32, kind="ExternalInput"),
        b2=nc.dram_tensor("b2", [H], fp32, kind="ExternalInput"),
        W3=nc.dram_tensor("W3", [H, OUT3], fp32, kind="ExternalInput"),
        b3=nc.dram_tensor("b3", [OUT3], fp32, kind="ExternalInput"),
        out=nc.dram_tensor("out", [BL, N], fp32, kind="ExternalOutput"),
        logdet=nc.dram_tensor("logdet", [BL, N], fp32, kind="ExternalOutput"),
    )
    with tile.TileContext(nc) as tc:
        _emit(nc, tc, io)
    nc.compile()
    return nc


def _emit(nc, tc, io):
    from contextlib import ExitStack
    import concourse.bass as bass
    from concourse import mybir

    fp32 = mybir.dt.float32
    i32 = mybir.dt.int32
    AF = mybir.ActivationFunctionType
    OP = mybir.AluOpType
    AX = mybir.AxisListType

    TT = nc.vector.tensor_tensor
    TS = nc.vector.tensor_scalar
    STT = nc.vector.scalar_tensor_tensor
    fp32r = mybir.dt.float32r

    def mmr(out, lhsT, rhs, **kw):
        # fp32r (TF32-like) would be 4x faster on PE but requires rounding
        # every producer to reduced precision; keep exact fp32.
        nc.tensor.matmul(out, lhsT, rhs, **kw)

    ctx = ExitStack()
    with ctx:
        singles = ctx.enter_context(tc.tile_pool(name="singles", bufs=1))
        work = ctx.enter_context(tc.tile_pool(name="work", bufs=3))
        fin = ctx.enter_context(tc.tile_pool(name="fin", bufs=2))
        psum_mm = ctx.enter_context(tc.tile_pool(name="psum_mm", bufs=2, space="PSUM"))
        psum_acc = ctx.enter_context(tc.tile_pool(name="psum_acc", bufs=1, space="PSUM"))
        dscr = ctx.enter_context(tc.tile_pool(name="dscr", bufs=2, space="DRAM"))

        dma = nc.sync.dma_start

        cnt = [0]

        def ps_tile(p, f):
            cnt[0] += 1
            return psum_mm.tile([p, f], fp32, tag="ps", name=f"ps{cnt[0]}")

        # ===== iota-derived constant masks =====
        iota_i = singles.tile([128, 1], i32)
        nc.gpsimd.iota(iota_i, pattern=[[0, 1]], base=0, channel_multiplier=1)
        iota_f = singles.tile([128, 1], fp32)
        nc.vector.tensor_copy(iota_f, iota_i)

        bkf_i = singles.tile([128, 16, 8], i32)   # value b' at col (b'*8+m)
        nc.gpsimd.iota(bkf_i, pattern=[[1, 16], [0, 8]], base=0, channel_multiplier=0)
        bkf_f = singles.tile([128, 16, 8], fp32)
        nc.vector.tensor_copy(bkf_f, bkf_i)

        colf_i = singles.tile([128, 128], i32)    # value j at col j
        nc.gpsimd.iota(colf_i, pattern=[[1, 128]], base=0, channel_multiplier=0)
        colf_f = singles.tile([128, 128], fp32)
        nc.vector.tensor_copy(colf_f, colf_i)

        pmod_i = singles.tile([128, 1], i32)      # p % 16
        TS(pmod_i, iota_i, 15, None, OP.bitwise_and)
        pmod_f = singles.tile([128, 1], fp32)
        nc.vector.tensor_copy(pmod_f, pmod_i)

        ident = singles.tile([128, 128], fp32)    # identity matrix
        TS(ident, colf_f, iota_f, None, OP.is_equal)

        lhsT16 = singles.tile([16, 128], fp32)     # [b, b'*8+m] = (b'==b)
        TS(lhsT16, bkf_f.rearrange("p a b -> p (a b)")[:16], iota_f[:16], None,
           OP.is_equal)

        maskbb = singles.tile([128, 16, 8], fp32)  # [p, (b',m)] = (p%16==b')
        TS(maskbb, bkf_f, pmod_f, None, OP.is_equal)

        # per-group replication masks: repl[gg][p, (b',m)] = (p == 16gg+b')
        repl = singles.tile([128, GG, 16, 8], fp32)
        for g in range(GG):
            pg = work.tile([128, 1], fp32, tag="pg", name="pg")
            TS(pg, iota_f, float(-16 * g), None, OP.add)
            TS(repl[:, g, :, :], bkf_f, pg, None, OP.is_equal)

        ps_h16 = ps_tile(128, 16)
        nc.tensor.transpose(ps_h16, lhsT16, ident[:16, :16])
        H16 = singles.tile([128, 16], fp32)        # [p, b'] = (p//8==b')
        nc.scalar.copy(H16, ps_h16)

        # gsel[p, g] = (p//16 == g); gqsel[p, q] = (p//32 == q)
        pdiv16_i = singles.tile([128, 1], i32)
        TS(pdiv16_i, iota_i, 4, None, OP.arith_shift_right)
        pdiv16_f = singles.tile([128, 1], fp32)
        nc.vector.tensor_copy(pdiv16_f, pdiv16_i)
        col8_i = singles.tile([128, 8], i32)
        nc.gpsimd.iota(col8_i, pattern=[[1, 8]], base=0, channel_multiplier=0)
        col8_f = singles.tile([128, 8], fp32)
        nc.vector.tensor_copy(col8_f, col8_i)
        gsel = singles.tile([128, 8], fp32)
        TS(gsel, col8_f, pdiv16_f, None, OP.is_equal)

        pdiv32_i = singles.tile([128, 1], i32)
        TS(pdiv32_i, iota_i, 5, None, OP.arith_shift_right)
        pdiv32_f = singles.tile([128, 1], fp32)
        nc.vector.tensor_copy(pdiv32_f, pdiv32_i)
        gqsel = singles.tile([128, 4], fp32)
        TS(gqsel, col8_f[:, 0:4], pdiv32_f, None, OP.is_equal)

        # maskC[p, (go',pi',b')] = ((p//16)%2 == go') * (p%16 == b')
        pm2_i = singles.tile([128, 1], i32)
        TS(pm2_i, pdiv16_i, 1, None, OP.bitwise_and)
        pm2_f = singles.tile([128, 1], fp32)
        nc.vector.tensor_copy(pm2_f, pm2_i)
        gof_i = singles.tile([128, 2, 4, 16], i32)
        nc.gpsimd.iota(gof_i, pattern=[[1, 2], [0, 4], [0, 16]], base=0,
                       channel_multiplier=0)
        gof_f = singles.tile([128, 2, 4, 16], fp32)
        nc.vector.tensor_copy(gof_f, gof_i)
        bf2_i = singles.tile([128, 2, 4, 16], i32)
        nc.gpsimd.iota(bf2_i, pattern=[[0, 2], [0, 4], [1, 16]], base=0,
                       channel_multiplier=0)
        bf2_f = singles.tile([128, 2, 4, 16], fp32)
        nc.vector.tensor_copy(bf2_f, bf2_i)
        mgo = singles.tile([128, 2, 4, 16], fp32)
        TS(mgo, gof_f, pm2_f, None, OP.is_equal)
        maskC = singles.tile([128, 2, 4, 16], fp32)
        mb2 = singles.tile([128, 2, 4, 16], fp32)
        TS(mb2, bf2_f, pmod_f, None, OP.is_equal)
        TT(maskC, mgo, mb2, OP.mult)

        # ===== weights =====
        W1s = singles.tile([CD, H], fp32)
        dma(out=W1s, in_=io["W1"][:, :])
        W2s = [singles.tile([128, H], fp32, tag=f"w2_{i}", name=f"w2_{i}") for i in range(2)]
        W3s = [singles.tile([128, OUT3], fp32, tag=f"w3_{i}", name=f"w3_{i}") for i in range(2)]
        for i in range(2):
            dma(out=W2s[i], in_=io["W2"][i * 128:(i + 1) * 128, :])
            dma(out=W3s[i], in_=io["W3"][i * 128:(i + 1) * 128, :])
        b1t = singles.tile([128, 2], fp32)
        dma(out=b1t, in_=io["b1"].rearrange("(h p) -> p h", p=128))
        b2t = singles.tile([128, 2], fp32)
        dma(out=b2t, in_=io["b2"].rearrange("(h p) -> p h", p=128))
        b3t = singles.tile([OUT3, 1], fp32)
        dma(out=b3t, in_=io["b3"].rearrange("(o u) -> o u", u=1))

        # ===== y, xc =====
        y_sb = singles.tile([128, T, N], fp32)
        xc_sb = singles.tile([128, T, N], fp32)
        for t in range(T):
            dma(out=y_sb[:, t, :], in_=io["y"][t * 128:(t + 1) * 128, :])
        for t in range(T):
            nc.gpsimd.tensor_scalar(xc_sb[:, t, :], y_sb[:, t, :], -BOUND, BOUND,
                                    OP.max, OP.min)

        # ===== MLP =====
        condT = singles.tile([CD, BL], fp32)
        for t in range(T):
            csb = work.tile([128, CD], fp32, tag="cond", name="csb")
            dma(out=csb, in_=io["cond"][t * 128:(t + 1) * 128, :])
            ps = ps_tile(CD, 128)
            nc.tensor.transpose(ps, csb, ident)
            nc.scalar.copy(condT[:, t * 128:(t + 1) * 128], ps)

        h1 = [singles.tile([128, BL], fp32, tag=f"h1_{i}", name=f"h1_{i}") for i in range(2)]
        for half in range(2):
            ps = ps_tile(128, BL)
            mmr(ps, W1s[:, half * 128:(half + 1) * 128], condT,
                start=True, stop=True)
            nc.scalar.activation(h1[half], ps, AF.Relu, bias=b1t[:, half:half + 1])
        h2 = [singles.tile([128, BL], fp32, tag=f"h2_{i}", name=f"h2_{i}") for i in range(2)]
        for half in range(2):
            ps = ps_tile(128, BL)
            for kc in range(2):
                mmr(ps, W2s[kc][:, half * 128:(half + 1) * 128], h1[kc],
                    start=(kc == 0), stop=(kc == 1))
            nc.scalar.activation(h2[half], ps, AF.Relu, bias=b2t[:, half:half + 1])
        p_f = singles.tile([OUT3, BL], fp32)
        ps49 = ps_tile(OUT3, BL)
        for kc in range(2):
            mmr(ps49, W3s[kc], h2[kc], start=(kc == 0), stop=(kc == 1))
        nc.scalar.activation(p_f, ps49, AF.Identity, bias=b3t)

        pw = singles.tile([128, T, OUT3], fp32)   # p row-major
        for t in range(T):
            ps = ps_tile(128, OUT3)
            nc.tensor.transpose(ps, p_f[:, t * 128:(t + 1) * 128], ident[:OUT3, :OUT3])
            nc.scalar.copy(pw[:, t, :], ps)

        # ===== param pipeline =====
        un_w = pw[:, :, 0:K]
        un_h = pw[:, :, K:2 * K]
        un_d = pw[:, :, 2 * K:3 * K + 1]

        def softmax_w(un, mb, tag):
            mx = singles.tile([128, T], fp32, tag=f"mx{tag}", name=f"mx{tag}")
            nc.vector.tensor_reduce(mx, un, axis=AX.X, op=OP.max)
            nmx = singles.tile([128, T], fp32, tag=f"nmx{tag}", name=f"nmx{tag}")
            TS(nmx, mx, -1.0, None, OP.mult)
            ein = singles.tile([128, T, K], fp32, tag=f"ein{tag}", name=f"ein{tag}")
            for t in range(T):
                TS(ein[:, t, :], un[:, t, :], nmx[:, t:t + 1], None, OP.add)
            ex = singles.tile([128, T, K], fp32, tag=f"ex{tag}", name=f"ex{tag}")
            nc.scalar.activation(ex, ein, AF.Exp)
            sm = singles.tile([128, T], fp32, tag=f"sm{tag}", name=f"sm{tag}")
            nc.vector.tensor_reduce(sm, ex, axis=AX.X, op=OP.add)
            rs = singles.tile([128, T], fp32, tag=f"rs{tag}", name=f"rs{tag}")
            nc.vector.reciprocal(rs, sm)
            wd = singles.tile([128, T, K], fp32, tag=f"wd{tag}", name=f"wd{tag}")
            for t in range(T):
                TS(wd[:, t, :], ex[:, t, :], rs[:, t:t + 1], 2 * BOUND - K * mb,
                   OP.mult, OP.mult)
            TS(wd, wd, mb, None, OP.add)
            return wd

        widths = softmax_w(un_w, MBW, "w")
        heights = softmax_w(un_h, MBH, "h")

        zeros16 = singles.tile([128, K], fp32)
        nc.vector.memset(zeros16, 0.0)
        cumw = singles.tile([128, T, K + 1], fp32)
        cumh = singles.tile([128, T, K + 1], fp32)
        nc.vector.memset(cumw[:, :, 0:1], -BOUND)
        nc.vector.memset(cumh[:, :, 0:1], -BOUND)
        for t in range(T):
            nc.vector.tensor_tensor_scan(cumw[:, t, 1:], widths[:, t, :], zeros16,
                                         -BOUND, OP.add, OP.add)
            nc.vector.tensor_tensor_scan(cumh[:, t, 1:], heights[:, t, :], zeros16,
                                         -BOUND, OP.add, OP.add)

        # softplus(x) = max(x,0) + ln(1 + exp(-|x|)) (no Softplus table on TRN2)
        deriv = singles.tile([128, T, K + 1], fp32)
        absd = singles.tile([128, T, K + 1], fp32)
        nc.scalar.activation(absd, un_d, AF.Abs)
        end_ = singles.tile([128, T, K + 1], fp32)
        nc.scalar.activation(end_, absd, AF.Exp, scale=-1.0)
        l1p = singles.tile([128, T, K + 1], fp32)
        nc.scalar.activation(l1p, end_, AF.Ln, bias=1.0)
        rl = singles.tile([128, T, K + 1], fp32)
        TS(rl, un_d, 0.0, MD, OP.max, OP.add)
        TT(deriv, rl, l1p, OP.add)

        d0 = deriv[:, :, 0:K]
        d1 = deriv[:, :, 1:K + 1]
        y0 = cumh[:, :, 0:K]
        kx = cumw[:, :, 0:K]
        kx1 = cumw[:, :, 1:K + 1]

        def tmp(tag):
            return singles.tile([128, T, K], fp32, tag=tag, name=tag)

        iw = tmp("iw"); nc.vector.reciprocal(iw, widths)
        delta = tmp("delta"); TT(delta, heights, iw, OP.mult)
        rdelta = tmp("rdelta"); nc.vector.reciprocal(rdelta, delta)
        s = tmp("s")
        for t in range(T):
            TS(s[:, t, :], rdelta[:, t, :], delta[:, t, 0:1], None, OP.mult)
        sig = tmp("sig"); TT(sig, d0, d1, OP.add)
        STT(sig, delta, -2.0, sig, OP.mult, OP.add)
        sdelta = tmp("sdelta"); TT(sdelta, s, delta, OP.mult)
        ssig = tmp("ssig"); TT(ssig, s, sig, OP.mult)
        sh = tmp("sh"); TT(sh, s, heights, OP.mult)
        shd0 = tmp("shd0"); TT(shd0, sh, d0, OP.mult)
        t1 = tmp("t1"); TT(t1, y0, ssig, OP.mult)
        Nc1 = tmp("Nc1"); TT(Nc1, t1, shd0, OP.add)
        u1 = tmp("u1"); TT(u1, delta, d0, OP.subtract)
        u2 = tmp("u2"); TT(u2, sh, u1, OP.mult)
        Nc2 = tmp("Nc2"); TT(Nc2, u2, t1, OP.subtract)
        sd2 = tmp("sd2"); TT(sd2, sdelta, sdelta, OP.mult)
        Cc1 = tmp("Cc1"); STT(Cc1, sd2, 2.0, u1, OP.mult, OP.mult)
        Cc2 = tmp("Cc2"); TT(Cc2, sd2, sig, OP.mult)
        iw2 = tmp("iw2"); TT(iw2, iw, iw, OP.mult)

        # final coefs into one contiguous tile: coefcat[:, t, ci, k]
        # ci: 0=aN 1=bN 2=aD 3=bD 4=aC 5=bC 6=kx 7=kx1
        coefcat = singles.tile([128, T, 8, K], fp32)
        aN = coefcat[:, :, 0, :]; TT(aN, Nc2, iw2, OP.mult)
        bN = coefcat[:, :, 1, :]; TT(bN, Nc1, iw, OP.mult)
        aD = coefcat[:, :, 2, :]; STT(aD, ssig, -1.0, iw2, OP.mult, OP.mult)
        bD = coefcat[:, :, 3, :]; TT(bD, ssig, iw, OP.mult)
        aC = coefcat[:, :, 4, :]; TT(aC, Cc2, iw2, OP.mult)
        bC = coefcat[:, :, 5, :]; TT(bC, Cc1, iw, OP.mult)
        nc.vector.tensor_copy(coefcat[:, :, 6, :], kx)
        nc.vector.tensor_copy(coefcat[:, :, 7, :], kx1)

        # per-row constants, packed 4-wide (pi 3 = 0) for the cpk transform
        constcat4 = singles.tile([128, 4, T], fp32)
        nc.vector.memset(constcat4[:, 3, :], 0.0)
        constN = constcat4[:, 0, :]
        TT(constN, y0[:, :, 0], sdelta[:, :, 0], OP.mult)
        constD = constcat4[:, 1, :]
        nc.vector.tensor_copy(constD, sdelta[:, :, 0])
        constC = constcat4[:, 2, :]
        TT(constC, sd2[:, :, 0], d0[:, :, 0], OP.mult)

        if MODE == "t":
            for cst, b in ((constN, bN), (constD, bD), (constC, bC)):
                bx = tmp("bx"); TT(bx, b, kx, OP.mult)
                sbx = singles.tile([128, T], fp32, tag="sbx", name="sbx")
                nc.vector.tensor_reduce(sbx, bx, axis=AX.X, op=OP.add)
                TT(cst, cst, sbx, OP.subtract)

        # ===== repack coefficients to (b*8+m) partition layout, k = 8h+m ====
        # PACKN[p=(b*8+m), t, ci, h, g] = coefcat[16g+b, t, ci, 8h+m]
        # via PE: PACK = (coef-expand * maskbb)^T @ gsel  (contraction over
        # the 128 source rows; gsel selects the group).
        PACKN = singles.tile([128, T, 8, 2, GG], fp32)
        for t in range(T):
            psp = ps_tile(128, 128)
            for h in range(2):
                exbig = work.tile([128, 8, 16, 8], fp32, tag="exbig",
                                  name="exbig")
                in0 = coefcat[:, t, :, 8 * h:8 * h + 8].unsqueeze(2)\
                    .broadcast_to([128, 8, 16, 8])
                in1 = maskbb.unsqueeze(1).broadcast_to([128, 8, 16, 8])
                TT(exbig, in0, in1, OP.mult)
                for ci in range(8):
                    lhs = exbig[:, ci, :, :].rearrange("p a b -> p (a b)")
                    nc.tensor.matmul(psp[:, (ci * 2 + h) * 8:(ci * 2 + h) * 8 + 8],
                                     lhs, gsel, start=True, stop=True)
            nc.scalar.copy(
                PACKN[:, t, :, :, :].rearrange("p a b c -> p (a b c)"), psp)
        NEGKX = singles.tile([128, T, 2, GG], fp32)
        TS(NEGKX, PACKN[:, :, 6, :, :], -1.0, None, OP.mult)

        # cpk[p=(go*64+pi*16+b), t, gq] = const_pi[32gq+16go+b, t] (SACC layout)
        CPK = singles.tile([128, T, 4], fp32)
        psc = ps_tile(128, 16)
        for t in range(T):
            cE = work.tile([128, 2, 4, 16], fp32, tag="cE", name="cE")
            cin = constcat4[:, :, t].unsqueeze(1).unsqueeze(3)\
                .broadcast_to([128, 2, 4, 16])
            TT(cE, cin, maskC, OP.mult)
            nc.tensor.matmul(psc[:, t * 4:(t + 1) * 4],
                             cE.rearrange("p a b c -> p (a b c)"), gqsel,
                             start=True, stop=True)
        nc.scalar.copy(CPK.rearrange("p a b -> p (a b)"), psc)

        # lhsT mega: [128, T, 2, GG, 4, 16]; per (t,h,g) a contiguous
        # [4poly, 16b'] = 64-col block (poly 3 = zeros, pads po to 64 so
        # start=True initializes the full PSUM slot)
        LHS_L = singles.tile([128, T, 2, GG, 4, 16], fp32)
        LHS_Q = singles.tile([128, T, 2, GG, 4, 16], fp32)
        nc.vector.memset(LHS_L[:, :, :, :, 3, :], 0.0)
        nc.vector.memset(LHS_Q[:, :, :, :, 3, :], 0.0)
        for t in range(T):
            for h in range(2):
                for pi, (lin_c, sq_c) in enumerate(((1, 0), (3, 2), (5, 4))):
                    for dst, ci in ((LHS_L, lin_c), (LHS_Q, sq_c)):
                        csrc = PACKN[:, t, ci, h, :]  # [128, GG]
                        bcs = csrc.unsqueeze(2).broadcast_to([128, GG, 16])
                        h16b = H16.unsqueeze(1).broadcast_to([128, GG, 16])
                        TT(dst[:, t, h, :, pi, :], bcs, h16b, OP.mult)

        # ===== main loop =====
        for t in range(T):
            for c in range(CH):
                ACC = psum_acc.tile([128, 4 * 512], fp32, name="ACC")
                accv = ACC.rearrange("(go pb) (gq n) -> go pb gq n", pb=64, n=512)
                for g in range(GG):
                    xrep = psum_mm.tile([128, 512], fp32, tag="xrep", name="xrep")
                    mmr(xrep, repl[:, g, :, :].rearrange("p a b -> p (a b)"),
                        xc_sb[:, t, c * 512:(c + 1) * 512],
                        start=True, stop=True)
                    slot = ACC[(g % 2) * 64:(g % 2) * 64 + 64,
                               (g // 2) * 512:(g // 2) * 512 + 512]
                    for h in range(2):
                        tk = work.tile([128, 512], fp32, tag="tk", name="tk")
                        TS(tk, xrep, PACKN[:, t, 6, h, g:g + 1],
                           PACKN[:, t, 7, h, g:g + 1], OP.max, OP.min)
                        usq = work.tile([128, 512], fp32, tag="usq", name="usq")
                        nc.scalar.activation(usq, tk, AF.Square,
                                             bias=NEGKX[:, t, h, g:g + 1])
                        if MODE == "u":
                            ulin = work.tile([128, 512], fp32, tag="ulin",
                                             name="ulin")
                            TS(ulin, tk, NEGKX[:, t, h, g:g + 1], None, OP.add)
                            lin_rhs = ulin
                        else:
                            lin_rhs = tk
                        ll = LHS_L[:, t, h, g, :, :].rearrange("p a b -> p (a b)")
                        lq = LHS_Q[:, t, h, g, :, :].rearrange("p a b -> p (a b)")
                        mmr(slot, ll, lin_rhs, start=(h == 0), stop=False)
                        mmr(slot, lq, usq, start=False, stop=(h == 1))

                # PSUM -> SBUF with per-row consts folded in (DMA can't
                # read PSUM); copies split across ACT/DVE
                SACC = fin.tile([128, 4, 512], fp32, tag="SACC", name="SACC")
                for bank in range(4):
                    if bank != 1:
                        nc.scalar.activation(SACC[:, bank, :],
                                             ACC[:, bank * 512:(bank + 1) * 512],
                                             AF.Identity,
                                             bias=CPK[:, t, bank:bank + 1])
                    else:
                        TS(SACC[:, bank, :], ACC[:, bank * 512:(bank + 1) * 512],
                           CPK[:, t, bank:bank + 1], None, OP.add)
                # bounce through DRAM to un-interleave (poly, b) rows:
                # 6 scattered writes + 3 contiguous reads beat 24 direct DMAs
                D1 = dscr.tile([3, 128, 512], fp32, name="D1")
                for go in range(2):
                    for pi in range(3):
                        psrc = SACC[go * 64 + pi * 16:go * 64 + pi * 16 + 16, :, :]
                        dview = bass.AP(
                            tensor=D1.tensor,
                            offset=D1.offset + pi * 128 * 512 + go * 16 * 512,
                            ap=[[512, 16], [32 * 512, 4], [1, 512]])
                        dmax = dma if (go * 3 + pi) % 2 == 0 else nc.scalar.dma_start
                        dmax(out=dview, in_=psrc)
                polys = []
                for pi in range(3):
                    dstt = fin.tile([128, 512], fp32, tag=f"poly{pi}",
                                    name=f"poly{pi}")
                    dmax = dma if pi % 2 == 0 else nc.scalar.dma_start
                    dmax(out=dstt, in_=D1[pi, :, :])
                    polys.append(dstt)
                Np, Dp, Cp = polys

                # finale (ee/inz/outF/ldF on the otherwise-idle GPSIMD)
                ysl = y_sb[:, t, c * 512:(c + 1) * 512]
                xsl = xc_sb[:, t, c * 512:(c + 1) * 512]
                Cm = fin.tile([128, 512], fp32, tag="Cm", name="Cm")
                nc.gpsimd.tensor_scalar(Cm, Cp, 1e-12, None, OP.max)
                rD = fin.tile([128, 512], fp32, tag="rD", name="rD")
                nc.vector.reciprocal(rD, Dp)
                out0 = fin.tile([128, 512], fp32, tag="out0", name="out0")
                nc.gpsimd.tensor_tensor(out0, Np, rD, OP.mult)
                logD = fin.tile([128, 512], fp32, tag="logD", name="logD")
                nc.scalar.activation(logD, Dp, AF.Ln)
                logC = fin.tile([128, 512], fp32, tag="logC", name="logC")
                nc.scalar.activation(logC, Cm, AF.Ln)
                ld0 = fin.tile([128, 512], fp32, tag="ld0", name="ld0")
                STT(ld0, logD, -2.0, logC, OP.mult, OP.add)
                ee = fin.tile([128, 512], fp32, tag="ee", name="ee")
                nc.gpsimd.tensor_tensor(ee, ysl, xsl, OP.subtract)
                inz = fin.tile([128, 512], fp32, tag="inz", name="inz")
                nc.gpsimd.tensor_scalar(inz, ee, 0.0, None, OP.is_equal)
                outF = fin.tile([128, 512], fp32, tag="outF", name="outF")
                nc.gpsimd.tensor_tensor(outF, out0, ee, OP.add)
                ldF = fin.tile([128, 512], fp32, tag="ldF", name="ldF")
                nc.gpsimd.tensor_tensor(ldF, ld0, inz, OP.mult)
                dma(out=io["out"][t * 128:(t + 1) * 128, c * 512:(c + 1) * 512],
                    in_=outF)
                nc.scalar.dma_start(
                    out=io["logdet"][t * 128:(t + 1) * 128, c * 512:(c + 1) * 512],
                    in_=ldF)


def kernel(cond, y, W1, b1, W2, b2, W3, b3):
    _ensure_path()
    from concourse.bass_utils import run_bass_kernel_spmd

    if "nc" not in _CACHE:
        _CACHE["nc"] = _build_nc()
    nc = _CACHE["nc"]

    cond = np.ascontiguousarray(cond, np.float32)
    y = np.ascontiguousarray(y, np.float32)
    shared = dict(W1=np.ascontiguousarray(W1, np.float32),
                  b1=np.ascontiguousarray(b1, np.float32),
                  W2=np.ascontiguousarray(W2, np.float32),
                  b2=np.ascontiguousarray(b2, np.float32),
                  W3=np.ascontiguousarray(W3, np.float32),
                  b3=np.ascontiguousarray(b3, np.float32))
    in_maps = []
    for i in range(NCORES):
        sl = slice(i * BL, (i + 1) * BL)
        in_maps.append(dict(cond=cond[sl], y=y[sl], **shared))
    res = run_bass_kernel_spmd(nc, in_maps, core_ids=list(range(NCORES)))
    out = np.concatenate([r["out"] for r in res.results], axis=0)
    ld = np.concatenate([r["logdet"] for r in res.results], axis=0)
    return out, ld



# revision 47
# speedup vs baseline: 1.6290x; 1.6290x over previous
"""Trainium2 Bass kernel for nn_ConditionalSpline1DFlow (K=16 RQS flow).

Data-parallel over 8 cores (B=4096 -> 512 rows/core). Per core:
  1. Conditioner MLP on TensorE (feature-major).
  2. Spline params per row; rescale bin k's rational-quadratic by
     s_k = delta_0/delta_k so numerator N, denominator D and
     derivative-numerator C become globally CONTINUOUS piecewise
     quadratics in x.
  3. Evaluate N, D, C gather-free in the clipped-ramp basis
        P(x) = const + sum_k a_k*(t_k - x_k)^2 + b_k*(t_k - x_k),
        t_k = clip(x, x_k, x_{k+1})
     on TensorE: rows packed (b*16+k) so one [128, 24] matmul contracts
     all 16 bins x 3 polys for 8 batch rows at once; PSUM accumulates the
     (linear, square) stream pair.
  4. out = N/D + (y - clip(y)); logdet = (ln C - 2 ln D) * (y == clip(y)).
"""
import sys
import numpy as np

K = 16
BOUND = 5.0
MBW = 1e-3
MBH = 1e-3
MD = 1e-3
B_FULL, N = 4096, 1024
CD, H = 64, 256
OUT3 = 3 * K + 1
NCORES = 8
BL = B_FULL // NCORES   # 512 rows per core
T = BL // 128           # 4 partition tiles
G = 128 // 8            # (unused) 8-row groups
GG = 128 // 16          # 8 groups of 16 rows per tile
CH = N // 512           # 2 free-dim chunks

# "t": stream clipped-t w/ folded consts; "u": stream t - x_k.
# "u" is required with fp32r matmuls: the t-basis linear stream telescopes
# large saturated-bin terms (30-100x cancellation), amplifying fp32r's
# ~2^-13 stream rounding into 1e-2..0.4 output error. u stays in [0, w_k].
MODE = "u"

_CACHE = {}


def _ensure_path():
    for p in ("/opt/trn_rl_repo",):
        if p not in sys.path:
            sys.path.insert(0, p)


def _build_nc():
    _ensure_path()
    import concourse.bacc as bacc
    import concourse.tile as tile
    from concourse import mybir

    fp32 = mybir.dt.float32
    nc = bacc.Bacc("TRN2", target_bir_lowering=False, debug=False)

    io = dict(
        cond=nc.dram_tensor("cond", [BL, CD], fp32, kind="ExternalInput"),
        y=nc.dram_tensor("y", [BL, N], fp32, kind="ExternalInput"),
        W1=nc.dram_tensor("W1", [CD, H], fp32, kind="ExternalInput"),
        b1=nc.dram_tensor("b1", [H], fp32, kind="ExternalInput"),
        W2=nc.dram_tensor("W2", [H, H], fp32, kind="ExternalInput"),
        b2=nc.dram_tensor("b2", [H], fp32, kind="ExternalInput"),
        W3=nc.dram_tensor("W3", [H, OUT3], fp32, kind="ExternalInput"),
        b3=nc.dram_tensor("b3", [OUT3], fp32, kind="ExternalInput"),
        out=nc.dram_tensor("out", [BL, N], fp32, kind="ExternalOutput"),
        logdet=nc.dram_tensor("logdet", [BL, N], fp32, kind="ExternalOutput"),
    )
    with tile.TileContext(nc) as tc:
        _emit(nc, tc, io)
    nc.compile()
    return nc


def _emit(nc, tc, io):
    from contextlib import ExitStack
    import concourse.bass as bass
    from concourse import mybir

    fp32 = mybir.dt.float32
    i32 = mybir.dt.int32
    AF = mybir.ActivationFunctionType
    OP = mybir.AluOpType
    AX = mybir.AxisListType

    TT = nc.vector.tensor_tensor
    TS = nc.vector.tensor_scalar
    STT = nc.vector.scalar_tensor_tensor
    fp32r = mybir.dt.float32r

    import os
    use_r = os.environ.get("KBISECT", "") != "fp32"

    def mmr(out, lhsT, rhs, **kw):
        # fp32r: 1 cycle/row vs 4 for fp32 (~1.6e-4 rel err measured on HW).
        # Operand tiles are allocated float32r so their producers round.
        assert lhsT.dtype == fp32r and rhs.dtype == fp32r, (lhsT.dtype, rhs.dtype)
        if use_r:
            nc.tensor.matmul(out, lhsT, rhs, **kw)
        else:
            nc.tensor.matmul(out, lhsT.bitcast(fp32), rhs.bitcast(fp32), **kw)

    ctx = ExitStack()
    with ctx:
        singles = ctx.enter_context(tc.tile_pool(name="singles", bufs=1))
        work = ctx.enter_context(tc.tile_pool(name="work", bufs=3))
        fin = ctx.enter_context(tc.tile_pool(name="fin", bufs=2))
        psum_mm = ctx.enter_context(tc.tile_pool(name="psum_mm", bufs=2, space="PSUM"))
        psum_acc = ctx.enter_context(tc.tile_pool(name="psum_acc", bufs=3, space="PSUM"))
        dscr = ctx.enter_context(tc.tile_pool(name="dscr", bufs=2, space="DRAM"))
        scratch = ctx.enter_context(tc.tile_pool(name="scratch", bufs=1))

        dma = nc.sync.dma_start

        cnt = [0]

        def ps_tile(p, f):
            cnt[0] += 1
            return psum_mm.tile([p, f], fp32, tag="ps", name=f"ps{cnt[0]}")

        # ===== iota-derived constant masks =====
        iota_i = singles.tile([128, 1], i32)
        nc.gpsimd.iota(iota_i, pattern=[[0, 1]], base=0, channel_multiplier=1)
        iota_f = singles.tile([128, 1], fp32)
        nc.vector.tensor_copy(iota_f, iota_i)

        bkf_i = singles.tile([128, 16, 8], i32)   # value b' at col (b'*8+m)
        nc.gpsimd.iota(bkf_i, pattern=[[1, 16], [0, 8]], base=0, channel_multiplier=0)
        bkf_f = singles.tile([128, 16, 8], fp32)
        nc.vector.tensor_copy(bkf_f, bkf_i)

        colf_i = singles.tile([128, 128], i32)    # value j at col j
        nc.gpsimd.iota(colf_i, pattern=[[1, 128]], base=0, channel_multiplier=0)
        colf_f = singles.tile([128, 128], fp32)
        nc.vector.tensor_copy(colf_f, colf_i)

        pmod_i = singles.tile([128, 1], i32)      # p % 16
        TS(pmod_i, iota_i, 15, None, OP.bitwise_and)
        pmod_f = singles.tile([128, 1], fp32)
        nc.vector.tensor_copy(pmod_f, pmod_i)

        ident = singles.tile([128, 128], fp32)    # identity matrix
        TS(ident, colf_f, iota_f, None, OP.is_equal)

        lhsT16 = singles.tile([16, 128], fp32)     # [b, b'*8+m] = (b'==b)
        TS(lhsT16, bkf_f.rearrange("p a b -> p (a b)")[:16], iota_f[:16], None,
           OP.is_equal)

        maskbb = singles.tile([128, 16, 8], fp32)  # [p, (b',m)] = (p%16==b')
        TS(maskbb, bkf_f, pmod_f, None, OP.is_equal)

        # per-group replication masks: repl[gg][p, (b',m)] = (p == 16gg+b')
        repl = singles.tile([128, GG, 16, 8], fp32r)
        for g in range(GG):
            pg = work.tile([128, 1], fp32, tag="pg", name="pg")
            TS(pg, iota_f, float(-16 * g), None, OP.add)
            TS(repl[:, g, :, :], bkf_f, pg, None, OP.is_equal)

        ps_h16 = ps_tile(128, 16)
        nc.tensor.transpose(ps_h16, lhsT16, ident[:16, :16])
        H16 = singles.tile([128, 16], fp32)        # [p, b'] = (p//8==b')
        nc.scalar.copy(H16, ps_h16)

        # gsel[p, g] = (p//16 == g); gqsel[p, q] = (p//32 == q)
        pdiv16_i = singles.tile([128, 1], i32)
        TS(pdiv16_i, iota_i, 4, None, OP.arith_shift_right)
        pdiv16_f = singles.tile([128, 1], fp32)
        nc.vector.tensor_copy(pdiv16_f, pdiv16_i)
        col8_i = singles.tile([128, 8], i32)
        nc.gpsimd.iota(col8_i, pattern=[[1, 8]], base=0, channel_multiplier=0)
        col8_f = singles.tile([128, 8], fp32)
        nc.vector.tensor_copy(col8_f, col8_i)
        gsel = singles.tile([128, 8], fp32)
        TS(gsel, col8_f, pdiv16_f, None, OP.is_equal)

        pdiv32_i = singles.tile([128, 1], i32)
        TS(pdiv32_i, iota_i, 5, None, OP.arith_shift_right)
        pdiv32_f = singles.tile([128, 1], fp32)
        nc.vector.tensor_copy(pdiv32_f, pdiv32_i)
        gqsel = singles.tile([128, 4], fp32)
        TS(gqsel, col8_f[:, 0:4], pdiv32_f, None, OP.is_equal)

        # maskC[p, (go',pi',b')] = ((p//16)%2 == go') * (p%16 == b')
        pm2_i = singles.tile([128, 1], i32)
        TS(pm2_i, pdiv16_i, 1, None, OP.bitwise_and)
        pm2_f = singles.tile([128, 1], fp32)
        nc.vector.tensor_copy(pm2_f, pm2_i)
        gof_i = singles.tile([128, 2, 4, 16], i32)
        nc.gpsimd.iota(gof_i, pattern=[[1, 2], [0, 4], [0, 16]], base=0,
                       channel_multiplier=0)
        gof_f = singles.tile([128, 2, 4, 16], fp32)
        nc.vector.tensor_copy(gof_f, gof_i)
        bf2_i = singles.tile([128, 2, 4, 16], i32)
        nc.gpsimd.iota(bf2_i, pattern=[[0, 2], [0, 4], [1, 16]], base=0,
                       channel_multiplier=0)
        bf2_f = singles.tile([128, 2, 4, 16], fp32)
        nc.vector.tensor_copy(bf2_f, bf2_i)
        mgo = singles.tile([128, 2, 4, 16], fp32)
        TS(mgo, gof_f, pm2_f, None, OP.is_equal)
        maskC = singles.tile([128, 2, 4, 16], fp32)
        mb2 = singles.tile([128, 2, 4, 16], fp32)
        TS(mb2, bf2_f, pmod_f, None, OP.is_equal)
        TT(maskC, mgo, mb2, OP.mult)

        # ===== weights =====
        W1s = singles.tile([CD, H], fp32)
        dma(out=W1s, in_=io["W1"][:, :])
        W2s = [singles.tile([128, H], fp32, tag=f"w2_{i}", name=f"w2_{i}") for i in range(2)]
        W3s = [singles.tile([128, OUT3], fp32, tag=f"w3_{i}", name=f"w3_{i}") for i in range(2)]
        for i in range(2):
            dma(out=W2s[i], in_=io["W2"][i * 128:(i + 1) * 128, :])
            dma(out=W3s[i], in_=io["W3"][i * 128:(i + 1) * 128, :])
        # fp32r-rounded weight copies (matmul operands need rounded producers)
        W1r = singles.tile([CD, H], fp32r, tag="w1r", name="w1r")
        nc.scalar.copy(W1r, W1s)
        W2r = [singles.tile([128, H], fp32r, tag=f"w2r_{i}", name=f"w2r_{i}") for i in range(2)]
        W3r = [singles.tile([128, OUT3], fp32r, tag=f"w3r_{i}", name=f"w3r_{i}") for i in range(2)]
        for i in range(2):
            nc.scalar.copy(W2r[i], W2s[i])
            nc.scalar.copy(W3r[i], W3s[i])
        b1t = singles.tile([128, 2], fp32)
        dma(out=b1t, in_=io["b1"].rearrange("(h p) -> p h", p=128))
        b2t = singles.tile([128, 2], fp32)
        dma(out=b2t, in_=io["b2"].rearrange("(h p) -> p h", p=128))
        b3t = singles.tile([OUT3, 1], fp32)
        dma(out=b3t, in_=io["b3"].rearrange("(o u) -> o u", u=1))

        # ===== y -> xc (exact fp32), xcr (fp32r hi) + xlo (residual), ee =====
        # xrep = repl@xcr + repl@xlo reconstructs x to fp32 precision on PE;
        # ee = y - clip(y) precomputed (exact) for the outside-mask finale.
        xcr_sb = singles.tile([128, T, N], fp32r)
        xlo_sb = singles.tile([128, T, N], fp32r)
        ee_sb = singles.tile([128, T, N], fp32)
        yt_s = singles.tile([128, N], fp32)
        xce_s = singles.tile([128, N], fp32)
        for t in range(T):
            dma(out=yt_s, in_=io["y"][t * 128:(t + 1) * 128, :])
            nc.gpsimd.tensor_scalar(xce_s, yt_s, -BOUND, BOUND, OP.max, OP.min)
            TS(xcr_sb[:, t, :], xce_s, 0.0, None, OP.add)
            TT(xlo_sb[:, t, :], xce_s, xcr_sb[:, t, :].bitcast(fp32),
               OP.subtract)
            nc.gpsimd.tensor_tensor(ee_sb[:, t, :], yt_s, xce_s, OP.subtract)

        # ===== MLP =====
        condT = singles.tile([CD, BL], fp32r)
        for t in range(T):
            csb = work.tile([128, CD], fp32, tag="cond", name="csb")
            dma(out=csb, in_=io["cond"][t * 128:(t + 1) * 128, :])
            ps = ps_tile(CD, 128)
            nc.tensor.transpose(ps, csb, ident)
            nc.scalar.copy(condT[:, t * 128:(t + 1) * 128], ps)

        h1 = [singles.tile([128, BL], fp32r, tag=f"h1_{i}", name=f"h1_{i}") for i in range(2)]
        for half in range(2):
            ps = ps_tile(128, BL)
            mmr(ps, W1r[:, half * 128:(half + 1) * 128], condT,
                start=True, stop=True)
            nc.scalar.activation(h1[half], ps, AF.Relu, bias=b1t[:, half:half + 1])
        h2 = [singles.tile([128, BL], fp32r, tag=f"h2_{i}", name=f"h2_{i}") for i in range(2)]
        for half in range(2):
            ps = ps_tile(128, BL)
            for kc in range(2):
                mmr(ps, W2r[kc][:, half * 128:(half + 1) * 128], h1[kc],
                    start=(kc == 0), stop=(kc == 1))
            nc.scalar.activation(h2[half], ps, AF.Relu, bias=b2t[:, half:half + 1])
        p_f = singles.tile([OUT3, BL], fp32)
        ps49 = ps_tile(OUT3, BL)
        for kc in range(2):
            mmr(ps49, W3r[kc], h2[kc], start=(kc == 0), stop=(kc == 1))
        nc.scalar.activation(p_f, ps49, AF.Identity, bias=b3t)

        pw = singles.tile([128, T, OUT3], fp32)   # p row-major
        for t in range(T):
            ps = ps_tile(128, OUT3)
            nc.tensor.transpose(ps, p_f[:, t * 128:(t + 1) * 128], ident[:OUT3, :OUT3])
            nc.scalar.copy(pw[:, t, :], ps)

        # ===== param pipeline =====
        un_w = pw[:, :, 0:K]
        un_h = pw[:, :, K:2 * K]
        un_d = pw[:, :, 2 * K:3 * K + 1]

        def softmax_w(un, mb, tag):
            mx = singles.tile([128, T], fp32, tag=f"mx{tag}", name=f"mx{tag}")
            nc.vector.tensor_reduce(mx, un, axis=AX.X, op=OP.max)
            nmx = singles.tile([128, T], fp32, tag=f"nmx{tag}", name=f"nmx{tag}")
            TS(nmx, mx, -1.0, None, OP.mult)
            ein = singles.tile([128, T, K], fp32, tag=f"ein{tag}", name=f"ein{tag}")
            for t in range(T):
                TS(ein[:, t, :], un[:, t, :], nmx[:, t:t + 1], None, OP.add)
            ex = singles.tile([128, T, K], fp32, tag=f"ex{tag}", name=f"ex{tag}")
            nc.scalar.activation(ex, ein, AF.Exp)
            sm = singles.tile([128, T], fp32, tag=f"sm{tag}", name=f"sm{tag}")
            nc.vector.tensor_reduce(sm, ex, axis=AX.X, op=OP.add)
            rs = singles.tile([128, T], fp32, tag=f"rs{tag}", name=f"rs{tag}")
            nc.vector.reciprocal(rs, sm)
            wd = singles.tile([128, T, K], fp32, tag=f"wd{tag}", name=f"wd{tag}")
            for t in range(T):
                TS(wd[:, t, :], ex[:, t, :], rs[:, t:t + 1], 2 * BOUND - K * mb,
                   OP.mult, OP.mult)
            TS(wd, wd, mb, None, OP.add)
            return wd

        widths = softmax_w(un_w, MBW, "w")
        heights = softmax_w(un_h, MBH, "h")

        zeros16 = singles.tile([128, K], fp32)
        nc.vector.memset(zeros16, 0.0)
        cumw = singles.tile([128, T, K + 1], fp32)
        cumh = singles.tile([128, T, K + 1], fp32)
        nc.vector.memset(cumw[:, :, 0:1], -BOUND)
        nc.vector.memset(cumh[:, :, 0:1], -BOUND)
        for t in range(T):
            nc.vector.tensor_tensor_scan(cumw[:, t, 1:], widths[:, t, :], zeros16,
                                         -BOUND, OP.add, OP.add)
            nc.vector.tensor_tensor_scan(cumh[:, t, 1:], heights[:, t, :], zeros16,
                                         -BOUND, OP.add, OP.add)

        # softplus(x) = max(x,0) + ln(1 + exp(-|x|)) (no Softplus table on TRN2)
        deriv = singles.tile([128, T, K + 1], fp32)
        absd = singles.tile([128, T, K + 1], fp32)
        nc.scalar.activation(absd, un_d, AF.Abs)
        end_ = singles.tile([128, T, K + 1], fp32)
        nc.scalar.activation(end_, absd, AF.Exp, scale=-1.0)
        l1p = singles.tile([128, T, K + 1], fp32)
        nc.scalar.activation(l1p, end_, AF.Ln, bias=1.0)
        rl = singles.tile([128, T, K + 1], fp32)
        TS(rl, un_d, 0.0, MD, OP.max, OP.add)
        TT(deriv, rl, l1p, OP.add)

        d0 = deriv[:, :, 0:K]
        d1 = deriv[:, :, 1:K + 1]
        y0 = cumh[:, :, 0:K]
        kx = cumw[:, :, 0:K]
        kx1 = cumw[:, :, 1:K + 1]

        def tmp(tag):
            return singles.tile([128, T, K], fp32, tag=tag, name=tag)

        # Centered-theta basis: streams th = clip((x-kx)/w, 0, 1) (saturates
        # to EXACTLY 1.0 in fp32r) and vc = (th-1/2)^2 (EXACTLY 0.25 at both
        # ends -> inactive/saturated quad contributions cancel against the
        # const fold; saturated increment is carried by B = S alone).
        #   P(x) = const' + sum_k A_k*vc_k + B_k*th_k
        #   A_N = s[h(d-d0) - y0*sig], B_N = delta0*h
        #   A_D = -s*sig,              B_D = 0
        #   A_G = delta0^2*sig,        B_G = delta0^2*(d1-d0)
        #   const' = const - 0.25*sum_k fp32r(A_k);  ld = lnG - 2 lnD
        weff = tmp("weff"); TT(weff, kx1, kx, OP.subtract)
        iw = tmp("iw"); nc.vector.reciprocal(iw, weff)
        # nudge 1/w up so saturated (x-kx)*rwc >= 1 always hits the literal
        # 1.0 clamp exactly, even if fp32r storage truncates
        TS(iw, iw, 1.0 + 2.0 ** -22, None, OP.mult)
        delta = tmp("delta"); TT(delta, heights, iw, OP.mult)
        rdelta = tmp("rdelta"); nc.vector.reciprocal(rdelta, delta)
        dl0b = delta[:, :, 0:1].broadcast_to([128, T, K])
        s = tmp("s"); TT(s, rdelta, dl0b, OP.mult)
        sig = tmp("sig"); TT(sig, d0, d1, OP.add)
        STT(sig, delta, -2.0, sig, OP.mult, OP.add)
        ssig = tmp("ssig"); TT(ssig, s, sig, OP.mult)
        sh = tmp("sh"); TT(sh, s, heights, OP.mult)
        t1 = tmp("t1"); TT(t1, y0, ssig, OP.mult)
        u1 = tmp("u1"); TT(u1, delta, d0, OP.subtract)
        u2 = tmp("u2"); TT(u2, sh, u1, OP.mult)
        dd = tmp("dd"); TT(dd, d1, d0, OP.subtract)
        d0sq = singles.tile([128, T], fp32, tag="d0sq", name="d0sq")
        TT(d0sq, delta[:, :, 0], delta[:, :, 0], OP.mult)
        d0sqb = d0sq.unsqueeze(2).broadcast_to([128, T, K])

        # coefcat[:, t, ci, k]; ci: 0=A_N 1=B_N 2=A_D 3=B_D(0) 4=A_G 5=B_G
        #                           6=kx 7=rwc
        coefcat = singles.tile([128, T, 8, K], fp32)
        AN = coefcat[:, :, 0, :]; TT(AN, u2, t1, OP.subtract)
        BN = coefcat[:, :, 1, :]; TT(BN, heights, dl0b, OP.mult)
        AD = coefcat[:, :, 2, :]; TS(AD, ssig, -1.0, None, OP.mult)
        TS(coefcat[:, :, 3, :], ssig, 0.0, None, OP.mult)    # B_D = 0
        AG = coefcat[:, :, 4, :]; TT(AG, sig, d0sqb, OP.mult)
        BG = coefcat[:, :, 5, :]; TT(BG, dd, d0sqb, OP.mult)
        nc.vector.tensor_copy(coefcat[:, :, 6, :], kx)
        nc.vector.tensor_copy(coefcat[:, :, 7, :], iw)

        # round A's to fp32r NOW (so the later lhsT rounding is idempotent)
        # and build the 0.25*sum(A~) const folds from the rounded values.
        Ar = singles.tile([128, T, K], fp32r, tag="Ar", name="Ar")
        folds = []
        for ci in (0, 2, 4):
            TS(Ar, coefcat[:, :, ci, :], 0.0, None, OP.add)
            nc.vector.tensor_copy(coefcat[:, :, ci, :], Ar.bitcast(fp32))
            fold = singles.tile([128, T], fp32, tag=f"fold{ci}",
                                name=f"fold{ci}")
            nc.vector.tensor_reduce(fold, Ar.bitcast(fp32), axis=AX.X,
                                    op=OP.add)
            folds.append(fold)

        neghalf = singles.tile([128, 1], fp32, tag="neghalf", name="neghalf")
        nc.vector.memset(neghalf, -0.5)

        # per-row constants, packed 4-wide (pi 3 = 0) for the cpk transform
        constcat4 = singles.tile([128, 4, T], fp32)
        nc.vector.memset(constcat4[:, 3, :], 0.0)
        constN = constcat4[:, 0, :]
        TS(constN, delta[:, :, 0], -BOUND, None, OP.mult)     # -5*delta0
        STT(constN, folds[0], -0.25, constN, OP.mult, OP.add)
        constD = constcat4[:, 1, :]
        STT(constD, folds[1], -0.25, delta[:, :, 0], OP.mult, OP.add)
        constG = constcat4[:, 2, :]
        TT(constG, d0sq, deriv[:, :, 0], OP.mult)             # delta0^2*d_0
        STT(constG, folds[2], -0.25, constG, OP.mult, OP.add)

        # ===== repack coefficients to (b*8+m) partition layout, k = 8h+m ====
        # PACKN[p=(b*8+m), t, ci, h, g] = coefcat[16g+b, t, ci, 8h+m]
        # via PE: PACK = (coef-expand * maskbb)^T @ gsel  (contraction over
        # the 128 source rows; gsel selects the group).
        PACKN = singles.tile([128, T, 8, 2, GG], fp32)
        for t in range(T):
            psp = ps_tile(128, 128)
            for h in range(2):
                exbig = scratch.tile([128, 8, 16, 8], fp32, tag="exbig",
                                     name="exbig")
                in0 = coefcat[:, t, :, 8 * h:8 * h + 8].unsqueeze(2)\
                    .broadcast_to([128, 8, 16, 8])
                in1 = maskbb.unsqueeze(1).broadcast_to([128, 8, 16, 8])
                TT(exbig, in0, in1, OP.mult)
                for ci in range(8):
                    lhs = exbig[:, ci, :, :].rearrange("p a b -> p (a b)")
                    nc.tensor.matmul(psp[:, (ci * 2 + h) * 8:(ci * 2 + h) * 8 + 8],
                                     lhs, gsel, start=True, stop=True)
            nc.scalar.copy(
                PACKN[:, t, :, :, :].rearrange("p a b c -> p (a b c)"), psp)


        # cpk[p=(go*64+pi*16+b), t, gq] = const_pi[32gq+16go+b, t] (SACC layout)
        CPK = singles.tile([128, T, 4], fp32)
        psc = ps_tile(128, 16)
        for t in range(T):
            cE = work.tile([128, 2, 4, 16], fp32, tag="cE", name="cE")
            cin = constcat4[:, :, t].unsqueeze(1).unsqueeze(3)\
                .broadcast_to([128, 2, 4, 16])
            TT(cE, cin, maskC, OP.mult)
            nc.tensor.matmul(psc[:, t * 4:(t + 1) * 4],
                             cE.rearrange("p a b c -> p (a b c)"), gqsel,
                             start=True, stop=True)
        nc.scalar.copy(CPK.rearrange("p a b -> p (a b)"), psc)

        # lhsT mega: fp32r slot matmuls must write PSUM partition base 0
        # (ISA col_grp rule) and the stationary AP must be one contiguous
        # free dim, so each matmul presents 128 cols: [block|zeros] (even g)
        # or [zeros|block] (odd g). Layout per (t,h): four 192-col pair cells
        # "R_2q Z_q R_2q+1"; even g=2q reads at q*192, odd g=2q+1 at q*192+64.
        PTH = (GG // 2) * 192          # cols per (t,h) = 768
        NBLK = T * 2 * PTH
        LHS_Lf = singles.tile([128, NBLK], fp32r, tag="lhsl", name="lhsl")
        LHS_Qf = singles.tile([128, NBLK], fp32r, tag="lhsq", name="lhsq")
        lhs_pitch = [LHS_Lf.ap[0][0], LHS_Qf.ap[0][0]]

        def lhs_block(fi, t, h, g, pi=0, shape=None):
            # AP over real block g of (t,h): [128, (q?), (par?), 16*len]
            flat = (LHS_Lf, LHS_Qf)[fi]
            off = (t * 2 + h) * PTH + (g // 2) * 192 + (g % 2) * 128 + pi * 16
            ap = [[lhs_pitch[fi], 128]] + (shape or [[1, 16]])
            return bass.AP(tensor=flat.tensor, offset=flat.offset + off, ap=ap)

        # zero strips Z_q (cols 64..128 of each cell) + poly-slot-3 zeros
        zin = colf_f[:, 0:64].unsqueeze(1).broadcast_to([128, 4, 64])
        zs16 = colf_f[:, 0:16].unsqueeze(1).unsqueeze(2)\
            .broadcast_to([128, 4, 2, 16])
        for fi in range(2):
            flat = (LHS_Lf, LHS_Qf)[fi]
            for t in range(T):
                for h in range(2):
                    zv = bass.AP(tensor=flat.tensor,
                                 offset=flat.offset + (t * 2 + h) * PTH + 64,
                                 ap=[[lhs_pitch[fi], 128], [192, 4], [1, 64]])
                    TS(zv, zin, 0.0, None, OP.mult)
                    # slot 3 of each real block (cols 48..64 within block)
                    z3 = bass.AP(tensor=flat.tensor,
                                 offset=flat.offset + (t * 2 + h) * PTH + 48,
                                 ap=[[lhs_pitch[fi], 128], [192, 4], [128, 2],
                                     [1, 16]])
                    TS(z3, zs16, 0.0, None, OP.mult)
        # slot mapping: L slots (pi0,1,2) <- ci (1, 3, 5); Q <- (0, 2, 4)
        for t in range(T):
            for h in range(2):
                for pi, (lin_c, sq_c) in enumerate(((1, 0), (3, 2), (5, 4))):
                    for fi, ci in ((0, lin_c), (1, sq_c)):
                        csrc = PACKN[:, t, ci, h, :]  # [128, GG]
                        c2 = csrc.rearrange("p (a b) -> p a b", b=2)
                        bcs = c2.unsqueeze(3).broadcast_to([128, 4, 2, 16])
                        h16b = H16.unsqueeze(1).unsqueeze(2)\
                            .broadcast_to([128, 4, 2, 16])
                        dst = lhs_block(fi, t, h, 0, pi,
                                        shape=[[192, 4], [128, 2], [1, 16]])
                        TT(dst, bcs, h16b, OP.mult)

        def padded_lhs(fi, t, h, g):
            # 128 contiguous cols: even g -> [R|Z], odd g -> [Z|R]
            flat = (LHS_Lf, LHS_Qf)[fi]
            off = (t * 2 + h) * PTH + (g // 2) * 192 + (g % 2) * 64
            return bass.AP(tensor=flat.tensor, offset=flat.offset + off,
                           ap=[[lhs_pitch[fi], 128], [1, 128]])

        # ===== main loop =====
        for t in range(T):
            for c in range(CH):
                # PSUM accumulation per 512-col bank (gq); bufs>1 on the ACC
                # tag lets iteration i+1 accumulate while i's banks drain.
                SACC = fin.tile([128, 4, 512], fp32, tag="SACC", name="SACC")
                for gq in range(4):
                    ACCb = psum_acc.tile([128, 512], fp32, tag="ACC",
                                         name="ACC")
                    for gg in range(2):
                        g = 2 * gq + gg
                        replg = repl[:, g, :, :].rearrange("p a b -> p (a b)")
                        xrep = psum_mm.tile([128, 512], fp32, tag="xrep",
                                            name="xrep")
                        mmr(xrep, replg,
                            xcr_sb[:, t, c * 512:(c + 1) * 512],
                            start=True, stop=False)
                        mmr(xrep, replg,
                            xlo_sb[:, t, c * 512:(c + 1) * 512],
                            start=False, stop=True)
                        for h in range(2):
                            # w_ = (x - kx)*rwc on DVE (PSUM + 2 ptr scalars)
                            # th = clip(w_, 0, 1): literal -> Pool-capable
                            # vc = (th - 1/2)^2: ACT Square, literal bias
                            w_ = work.tile([128, 512], fp32, tag="w_",
                                           name="w_")
                            TS(w_, xrep, PACKN[:, t, 6, h, g:g + 1],
                               PACKN[:, t, 7, h, g:g + 1],
                               OP.subtract, OP.mult)
                            th = work.tile([128, 512], fp32r, tag="th",
                                           name="th")
                            sel = (gq * 4 + 2 * gg + h) % 16
                            tse = nc.vector if sel < 3 else nc.gpsimd
                            tse.tensor_scalar(th, w_, 0.0, 1.0,
                                              OP.max, OP.min)
                            vc = work.tile([128, 512], fp32r, tag="vc",
                                           name="vc")
                            nc.scalar.activation(vc, th.bitcast(fp32),
                                                 AF.Square, bias=neghalf)
                            ll = padded_lhs(0, t, h, g)
                            lq = padded_lhs(1, t, h, g)
                            mmr(ACCb, ll, th,
                                start=(gg == 0 and h == 0), stop=False)
                            mmr(ACCb, lq, vc, start=False,
                                stop=(gg == 1 and h == 1))
                    # PSUM -> SBUF with per-row consts folded in (ACT; DVE
                    # is the bottleneck engine)
                    nc.scalar.activation(SACC[:, gq, :], ACCb, AF.Identity,
                                         bias=CPK[:, t, gq:gq + 1])
                # bounce through DRAM to un-interleave (poly, b) rows:
                # 2 scattered writes + 1 strided read, all issued from SP
                D1 = dscr.tile([3, 128, 512], fp32, name="D1")
                for go in range(2):
                    for pi in range(3):
                        psrc = SACC[go * 64 + pi * 16:go * 64 + pi * 16 + 16, :, :]
                        dview = bass.AP(
                            tensor=D1.tensor,
                            offset=D1.offset + pi * 128 * 512 + go * 16 * 512,
                            ap=[[512, 16], [32 * 512, 4], [1, 512]])
                        dma(out=dview, in_=psrc)
                P3 = fin.tile([128, 3, 512], fp32, tag="P3", name="P3")
                s3 = bass.AP(tensor=D1.tensor, offset=D1.offset,
                             ap=[[512, 128], [128 * 512, 3], [1, 512]])
                dma(out=P3, in_=s3)
                Np, Dp, Cp = P3[:, 0, :], P3[:, 1, :], P3[:, 2, :]

                # finale, in-place on the poly tiles:
                #   Np -> out0 -> outF ; Dp -> logD -> ld0 -> ldF ; Gp -> logG
                # ld = lnG - 2 lnD + 2 ln(delta0);  out = N/D + ee
                Gp = Cp
                eesl = ee_sb[:, t, c * 512:(c + 1) * 512]
                rD = fin.tile([128, 512], fp32, tag="rD", name="rD")
                nc.vector.reciprocal(rD, Dp)
                nc.scalar.activation(Dp, Dp, AF.Ln)       # logD
                nc.scalar.activation(Gp, Gp, AF.Ln)       # logG
                nc.gpsimd.tensor_tensor(Np, Np, rD, OP.mult)   # out0
                nc.gpsimd.tensor_tensor(Np, Np, eesl, OP.add)  # outF
                STT(Dp, Dp, -2.0, Gp, OP.mult, OP.add)    # ld0
                # inz reuses rD's buffer (rD consumed by out0 above)
                nc.gpsimd.tensor_scalar(rD, eesl, 0.0, None, OP.is_equal)
                nc.gpsimd.tensor_tensor(Dp, Dp, rD, OP.mult)  # ldF
                dma(out=io["out"][t * 128:(t + 1) * 128, c * 512:(c + 1) * 512],
                    in_=Np)
                dma(out=io["logdet"][t * 128:(t + 1) * 128,
                                     c * 512:(c + 1) * 512],
                    in_=Dp)


def kernel(cond, y, W1, b1, W2, b2, W3, b3):
    _ensure_path()
    from concourse.bass_utils import run_bass_kernel_spmd

    if "nc" not in _CACHE:
        _CACHE["nc"] = _build_nc()
    nc = _CACHE["nc"]

    cond = np.ascontiguousarray(cond, np.float32)
    y = np.ascontiguousarray(y, np.float32)
    shared = dict(W1=np.ascontiguousarray(W1, np.float32),
                  b1=np.ascontiguousarray(b1, np.float32),
                  W2=np.ascontiguousarray(W2, np.float32),
                  b2=np.ascontiguousarray(b2, np.float32),
                  W3=np.ascontiguousarray(W3, np.float32),
                  b3=np.ascontiguousarray(b3, np.float32))
    in_maps = []
    for i in range(NCORES):
        sl = slice(i * BL, (i + 1) * BL)
        in_maps.append(dict(cond=cond[sl], y=y[sl], **shared))
    res = run_bass_kernel_spmd(nc, in_maps, core_ids=list(range(NCORES)))
    out = np.concatenate([r["out"] for r in res.results], axis=0)
    ld = np.concatenate([r["logdet"] for r in res.results], axis=0)
    return out, ld

